# revision 8
# baseline (speedup 1.0000x reference)
"""Trainium2 Bass kernel for a single attention head.

Problem: B=8, S=2048, I=H=512, fp32 in/out, int32 mask.
  q = x @ W_q.T + b_q ; k = x @ W_k.T + b_k ; v = x @ W_v.T + b_v
  scores = (q @ k.T) / sqrt(H) ; scores[mask==0] = -2^31
  out = softmax(scores) @ v

Sharding: data-parallel over batch -- core c computes batch element c.
No collectives needed; 8 independent NeuronCores.

Per-core algorithm (all on one core, S=2048 seq, H=I=512):
 - transpose x (via PE) to x^T (I x S); transpose W_q/k/v to W^T (I x H)
 - Q^T = W_q^T.T @ x^T   (H on partitions), fused (+b_q)*inv_sqrt on DVE
 - K^T likewise (+b_k); V = x^T.T @ W_v^T (+b_v via a K=1 ones matmul)
 - scores^T tile (k x q) = K^T.T @ Q^T accumulated in PSUM, then the mask
   is ADDED into the same PSUM accumulation as BIG * mask^T via a matmul
   with lhsT=mask block (bf16) and rhs=BIG*I, so
       psum = q.k + BIG*mask
   and exp(psum - BIG) on ScalarE gives exp(q.k) where mask=1 and ~0
   (exp(s-100)) where mask=0.  Scores are O(3) here so no max-subtraction
   is needed for fp32 exp stability.
 - row sums D over k (partition axis) via ones-vector matmuls accumulated
   in PSUM; 1/D via PE transpose of the row + DVE reciprocal
 - out tile = E^T.T @ V accumulated over k chunks, scaled by 1/D on the
   ScalarE during the PSUM->SBUF copy, then DMA out.

Matmuls run in float32r (full fp32 storage; reduced-precision PE mode at
1 cycle/row vs 4 for strict fp32).
"""

import numpy as np

import concourse.bass as bass
import concourse.bacc as bacc
import concourse.mybir as mybir
import concourse.tile as tile
from concourse.bass_utils import run_bass_kernel_spmd
from concourse.masks import make_identity

F32 = mybir.dt.float32
F32R = mybir.dt.float32r
BF16 = mybir.dt.bfloat16
I32 = mybir.dt.int32
AF = mybir.ActivationFunctionType
ALU = mybir.AluOpType

B, S, I, H = 8, 2048, 512, 512
P = 128
NIC = I // P      # 4  i chunks (contraction of projections)
NHT = H // P      # 4  h tiles
NST = S // P      # 16 s tiles
NKT = S // P      # 16 k tiles
NQG = 4           # q groups of 512
QG = S // NQG     # 512
BIG = 100.0
INV = float(1.0 / np.sqrt(H))


def _r(ap):
    return ap.bitcast(F32R)


def _build_kernel():
    nc = bacc.Bacc("TRN2", target_bir_lowering=False, debug=False, num_devices=8)
    x_d = nc.declare_dram_parameter("input", [S, I], F32, isOutput=False)
    m_d = nc.declare_dram_parameter("mask", [S, S], I32, isOutput=False)
    wq_d = nc.declare_dram_parameter("W_q", [H, I], F32, isOutput=False)
    bq_d = nc.declare_dram_parameter("b_q", [H], F32, isOutput=False)
    wk_d = nc.declare_dram_parameter("W_k", [H, I], F32, isOutput=False)
    bk_d = nc.declare_dram_parameter("b_k", [H], F32, isOutput=False)
    wv_d = nc.declare_dram_parameter("W_v", [H, I], F32, isOutput=False)
    bv_d = nc.declare_dram_parameter("b_v", [H], F32, isOutput=False)
    out_d = nc.declare_dram_parameter("out", [S, H], F32, isOutput=True)

    with tile.TileContext(nc) as tc:
        _kernel_body(tc, x_d.ap(), m_d.ap(), wq_d.ap(), bq_d.ap(), wk_d.ap(),
                     bk_d.ap(), wv_d.ap(), bv_d.ap(), out_d.ap())
    nc.compile()
    return nc


def _kernel_body(tc, x_a, m_a, wq_a, bq_a, wk_a, bk_a, wv_a, bv_a, out_a):
    with tc.tile_pool(name="consts", bufs=1) as consts, \
         tc.tile_pool(name="persist", bufs=1) as persist:
        _kernel_inner(tc, consts, persist, x_a, m_a, wq_a, bq_a, wk_a, bk_a,
                      wv_a, bv_a, out_a)


def _kernel_inner(tc, consts, persist, x_a, m_a, wq_a, bq_a, wk_a, bk_a, wv_a,
                  bv_a, out_a):
    nc = tc.nc

    ident = consts.tile([P, P], F32)
    make_identity(nc, ident)
    bigi = consts.tile([P, P], BF16)
    nc.scalar.activation(out=bigi, in_=ident, func=AF.Copy, scale=BIG)
    negbig = consts.tile([P, 1], F32)
    nc.vector.memset(negbig, -BIG)
    ones_f32 = consts.tile([P, 1], F32)
    nc.vector.memset(ones_f32, 1.0)
    ones_row_f32 = consts.tile([1, P], F32)
    nc.vector.memset(ones_row_f32, 1.0)
    ones_col = consts.tile([P, 1], F32R)
    nc.vector.tensor_copy(out=ones_col, in_=ones_f32)
    ones_row = consts.tile([1, P], F32R)
    nc.vector.tensor_copy(out=ones_row, in_=ones_row_f32)
    one_one = consts.tile([1, 1], F32)
    nc.vector.memset(one_one, 1.0)
    bq_t = consts.tile([P, NHT], F32)
    nc.sync.dma_start(out=bq_t, in_=bq_a.rearrange("(t p) -> p t", p=P))
    bk_t = consts.tile([P, NHT], F32)
    nc.sync.dma_start(out=bk_t, in_=bk_a.rearrange("(t p) -> p t", p=P))
    bv_stage = consts.tile([1, H], F32)
    nc.sync.dma_start(out=bv_stage, in_=bv_a.rearrange("(a h) -> a h", a=1))
    bv_row = consts.tile([1, H], F32R)
    nc.vector.tensor_copy(out=bv_row, in_=bv_stage)

    # Persistent activations.
    xT = [persist.tile([P, S], F32R, tag=f"xT{i}", name=f"xT{i}") for i in range(NIC)]
    qT = [persist.tile([P, S], F32R, tag=f"qT{i}", name=f"qT{i}") for i in range(NHT)]
    kT = [persist.tile([P, S], F32R, tag=f"kT{i}", name=f"kT{i}") for i in range(NHT)]
    vt = [persist.tile([P, H], F32R, tag=f"v{i}", name=f"v{i}") for i in range(NST)]

    # ---- Phase B: load x/W and transpose on the PE ----
    with tc.tile_pool(name="wT_pool", bufs=1) as wT_pool:
        wT = {w: [wT_pool.tile([P, H], F32R, tag=f"wT{w}{i}", name=f"wT{w}{i}")
                  for i in range(NIC)]
              for w in ("q", "k", "v")}

        with tc.tile_pool(name="stage", bufs=8) as stage, \
             tc.tile_pool(name="tp_ps", bufs=2, space="PSUM") as tp_ps:
            # x^T: load 4 s-tiles at a time; transpose each 128x128 block of
            # x (s-part, i-free) -> (i-part, s-free)
            for stg in range(NST // 4):
                xin = []
                for j in range(4):
                    st = stg * 4 + j
                    t = stage.tile([P, I], F32, tag="xin", name="xin")
                    nc.sync.dma_start(out=t, in_=x_a[st * P:(st + 1) * P, :])
                    xin.append(t)
                for ic in range(NIC):
                    ps = tp_ps.tile([P, 4 * P], F32, tag="tp", name="tp")
                    for j in range(4):
                        nc.tensor.transpose(
                            out=ps[:, j * P:(j + 1) * P],
                            in_=xin[j][:, ic * P:(ic + 1) * P],
                            identity=ident)
                    nc.any.tensor_copy(
                        out=xT[ic][:, stg * 4 * P:(stg + 1) * 4 * P], in_=ps)
            # W^T
            for wname, wa in (("q", wq_a), ("k", wk_a), ("v", wv_a)):
                win = []
                for ht in range(NHT):
                    t = stage.tile([P, I], F32, tag="xin", name="win")
                    nc.sync.dma_start(out=t, in_=wa[ht * P:(ht + 1) * P, :])
                    win.append(t)
                for ic in range(NIC):
                    ps = tp_ps.tile([P, 4 * P], F32, tag="tp", name="tp")
                    for ht in range(NHT):
                        nc.tensor.transpose(
                            out=ps[:, ht * P:(ht + 1) * P],
                            in_=win[ht][:, ic * P:(ic + 1) * P],
                            identity=ident)
                    nc.any.tensor_copy(out=wT[wname][ic], in_=ps)

        # ---- Phase C: projections ----
        with tc.tile_pool(name="proj_ps", bufs=4, space="PSUM") as proj_ps:
            for ht in range(NHT):
                for sc in range(NQG):
                    ps = proj_ps.tile([P, QG], F32, tag="proj", name="proj")
                    for ic in range(NIC):
                        nc.tensor.matmul(
                            ps, lhsT=wT["q"][ic][:, ht * P:(ht + 1) * P],
                            rhs=xT[ic][:, sc * QG:(sc + 1) * QG],
                            start=(ic == 0), stop=(ic == NIC - 1))
                    # (q + b_q) * inv_sqrt
                    nc.vector.tensor_scalar(
                        out=qT[ht][:, sc * QG:(sc + 1) * QG], in0=ps,
                        scalar1=bq_t[:, ht:ht + 1], scalar2=INV,
                        op0=ALU.add, op1=ALU.mult)
                for sc in range(NQG):
                    ps = proj_ps.tile([P, QG], F32, tag="proj", name="proj")
                    for ic in range(NIC):
                        nc.tensor.matmul(
                            ps, lhsT=wT["k"][ic][:, ht * P:(ht + 1) * P],
                            rhs=xT[ic][:, sc * QG:(sc + 1) * QG],
                            start=(ic == 0), stop=(ic == NIC - 1))
                    nc.vector.tensor_scalar(
                        out=kT[ht][:, sc * QG:(sc + 1) * QG], in0=ps,
                        scalar1=bk_t[:, ht:ht + 1], scalar2=None, op0=ALU.add)
            for st in range(NST):
                ps = proj_ps.tile([P, H], F32, tag="proj", name="proj")
                for ic in range(NIC):
                    nc.tensor.matmul(
                        ps, lhsT=xT[ic][:, st * P:(st + 1) * P],
                        rhs=wT["v"][ic], start=(ic == 0), stop=False)
                # + b_v broadcast over all rows: ones(1,128)^T @ b_v(1,512)
                nc.tensor.matmul(ps, lhsT=ones_row, rhs=bv_row,
                                 start=False, stop=True)
                nc.any.tensor_copy(out=vt[st], in_=ps)

    # ---- Phase D: attention, one q-group (512 q) at a time ----
    with tc.tile_pool(name="e_pool", bufs=1) as e_pool, \
         tc.tile_pool(name="mi_pool", bufs=4) as mi_pool, \
         tc.tile_pool(name="mb_pool", bufs=2) as mb_pool, \
         tc.tile_pool(name="qk_ps", bufs=3, space="PSUM") as qk_ps, \
         tc.tile_pool(name="d_ps", bufs=2, space="PSUM") as d_ps, \
         tc.tile_pool(name="pv_ps", bufs=2, space="PSUM") as pv_ps, \
         tc.tile_pool(name="rt_ps", bufs=1, space="PSUM") as rt_ps, \
         tc.tile_pool(name="small", bufs=2) as small, \
         tc.tile_pool(name="out_pool", bufs=3) as out_pool:
        for qg in range(NQG):
            dsum = d_ps.tile([1, QG], F32, tag="dsum", name="dsum")
            et = [None] * NKT
            for ktg in range(NKT // 4):
                # mask tiles for these 4 k-tiles: (128 q x 512 k) int32 -> bf16
                mb = []
                for qt in range(4):
                    mi = mi_pool.tile([P, 4 * P], I32, tag="mi", name="mi")
                    nc.sync.dma_start(
                        out=mi,
                        in_=m_a[qg * QG + qt * P: qg * QG + (qt + 1) * P,
                                ktg * 4 * P:(ktg + 1) * 4 * P])
                    mbt = mb_pool.tile([P, 4 * P], BF16, tag=f"mb{qt}",
                                       name=f"mb{qt}")
                    nc.vector.tensor_copy(out=mbt, in_=mi)
                    mb.append(mbt)
                for j in range(4):
                    kt = ktg * 4 + j
                    ps = qk_ps.tile([P, QG], F32, tag="qk", name="qk")
                    for hc in range(NHT):
                        nc.tensor.matmul(
                            ps, lhsT=kT[hc][:, kt * P:(kt + 1) * P],
                            rhs=qT[hc][:, qg * QG:(qg + 1) * QG],
                            start=(hc == 0), stop=False)
                    # += BIG * mask^T  (mask block is q-part, k-free)
                    for qt in range(4):
                        nc.tensor.matmul(
                            ps[:, qt * P:(qt + 1) * P],
                            lhsT=mb[qt][:, j * P:(j + 1) * P], rhs=bigi,
                            start=False, stop=(qt == 3))
                    e = e_pool.tile([P, QG], F32R, tag=f"e{kt}", name=f"e{kt}")
                    nc.scalar.activation(out=e, in_=ps, func=AF.Exp,
                                         bias=negbig, scale=1.0)
                    et[kt] = e
                    nc.tensor.matmul(dsum, lhsT=ones_col, rhs=e,
                                     start=(kt == 0), stop=(kt == NKT - 1))
            # 1/D, transposed to (128, 4) so q lands on partitions
            drow = small.tile([1, QG], F32, tag="drow", name="drow")
            nc.vector.tensor_copy(out=drow, in_=dsum)
            rt = rt_ps.tile([P, 4], F32, tag="rt", name="rt")
            for j in range(4):
                nc.tensor.matmul(rt[:, j:j + 1], lhsT=drow[:, j * P:(j + 1) * P],
                                 rhs=one_one, start=True, stop=True)
            rec = small.tile([P, 4], F32, tag="rec", name="rec")
            nc.vector.reciprocal(out=rec, in_=rt)
            # PV
            for qt in range(4):
                po = pv_ps.tile([P, H], F32, tag="pv", name="pv")
                for kt in range(NKT):
                    nc.tensor.matmul(
                        po, lhsT=et[kt][:, qt * P:(qt + 1) * P],
                        rhs=vt[kt], start=(kt == 0), stop=(kt == NKT - 1))
                ot = out_pool.tile([P, H], F32, tag="ot", name="ot")
                nc.scalar.activation(out=ot, in_=po, func=AF.Copy,
                                     scale=rec[:, qt:qt + 1])
                row0 = (qg * 4 + qt) * P
                nc.sync.dma_start(out=out_a[row0:row0 + P, :], in_=ot)


_NC = None


def _get_nc():
    global _NC
    if _NC is None:
        _NC = _build_kernel()
    return _NC


def kernel(**inputs):
    nc = _get_nc()
    in_maps = []
    for c in range(B):
        in_maps.append({
            "input": np.ascontiguousarray(inputs["input"][c]),
            "mask": np.ascontiguousarray(inputs["mask"][c]),
            "W_q": np.asarray(inputs["W_q"]),
            "b_q": np.asarray(inputs["b_q"]),
            "W_k": np.asarray(inputs["W_k"]),
            "b_k": np.asarray(inputs["b_k"]),
            "W_v": np.asarray(inputs["W_v"]),
            "b_v": np.asarray(inputs["b_v"]),
        })
    res = run_bass_kernel_spmd(nc, in_maps, core_ids=list(range(B)))
    return np.stack([res.results[c]["out"] for c in range(B)], axis=0)
